# revision 37
# baseline (speedup 1.0000x reference)
"""DisparityFusion Trainium2 kernel (8 NeuronCores, SPMD data-parallel).

Full inputs in, full output out. Sharding: core c handles batch b=c//4 and
output rows [64*(c%4), 64*(c%4)+64), with a 1-row halo of the disparity maps
computed locally from a 1-row halo of the cost volumes (zero-padded at image
boundaries, masked after the softmax-regression).

Per-core pipeline:
  Stage 1 (per branch): softmax-expectation over D=192.
    Cost slab loaded D-on-partitions (chunk A: d 0..127; chunk B: d 128..191
    packed two rows per 128 partitions). exp on ScalarE -> bf16. Sum(e) and
    Sum(e*d) via TensorE matmuls with block-diagonal stationary bf16 weights
    into one PSUM tile per 32-row block (s0 rows 0..31, s1 rows 32..63).
    Reciprocal-multiply on VectorE, mask boundary halo rows, DMA rows into a
    zero-bordered d_pad [66, 514].
  Stage 2: 3x3 conv (1->9ch) x3 branches + BN + ReLU -> 27 affinity gates,
    CSPN abs-sum normalization folded into the final division:
      out = sum_c aff_c*patch_c / (sum_c aff_c + 1e-8)   (aff >= 0 post-ReLU)
    Patches materialized once as IM[108, 16, 512] (partition = (9br+tap)*4+q,
    q = 16-row quarter) via 27 SBUF->SBUF f32->f32r cast DMAs from d_pad; the
    conv is one [108,108] f32r matmul per 512-pixel chunk; channel sums via
    ones-pattern matmuls (S in f32r, A in bf16).
"""

import sys

sys.path.insert(0, "/opt/trn_rl_repo")

from contextlib import ExitStack

import numpy as np
import ml_dtypes

import concourse.bass as bass
import concourse.bacc as bacc
import concourse.tile as tile
from concourse import mybir
from concourse import bass_utils

B, D, H, W = 2, 192, 256, 512
N_CORES = 8
SLAB = 64            # output rows per core
SLABP = SLAB + 2     # slab + 1-row halo each side
BLOCKS = [(0, 32), (32, 32), (64, 2)]  # (r0, nr) over the 66 slab rows

F32 = mybir.dt.float32
F16 = mybir.dt.float16
F32R = mybir.dt.float32r
BF16 = mybir.dt.bfloat16
BF16_NP = ml_dtypes.bfloat16


def _build_nc():
    nc = bacc.Bacc(
        "TRN2",
        target_bir_lowering=False,
        debug=False,
        enable_asserts=False,
        num_devices=N_CORES,
    )

    xs = [
        nc.dram_tensor(f"x{i}", [128, SLABP, W], F16, kind="ExternalInput").ap()
        for i in (1, 2, 3)
    ]
    xbs = [
        nc.dram_tensor(f"xb{i}", [128, SLABP // 2, W], F16, kind="ExternalInput").ap()
        for i in (1, 2, 3)
    ]
    la_d = nc.dram_tensor("la", [128, 32, 64], F16, kind="ExternalInput").ap()
    lb_d = nc.dram_tensor("lb", [128, 16, 64], F16, kind="ExternalInput").ap()
    wc_d = nc.dram_tensor("wc", [108, 108], F32, kind="ExternalInput").ap()
    ws_d = nc.dram_tensor("ws", [108, 4], F32, kind="ExternalInput").ap()
    bv_d = nc.dram_tensor("bv", [108, 1], F32, kind="ExternalInput").ap()
    m3_d = nc.dram_tensor("m3", [32, 3], F32, kind="ExternalInput").ap()
    z1_d = nc.dram_tensor("z1", [1, 1], F32, kind="ExternalInput").ap()
    out_d = nc.dram_tensor("out", [SLAB, W], F32, kind="ExternalOutput").ap()

    with tile.TileContext(nc) as tc, ExitStack() as ctx:
        consts = ctx.enter_context(tc.tile_pool(name="consts", bufs=1))
        dpads = ctx.enter_context(tc.tile_pool(name="dpads", bufs=1))

        la = consts.tile([128, 32, 64], F16, tag="la")
        nc.gpsimd.dma_start(la[:], la_d[:])
        lb = consts.tile([128, 16, 64], F16, tag="lb")
        nc.gpsimd.dma_start(lb[:], lb_d[:])
        wc = consts.tile([108, 108], F16, tag="wc")
        nc.gpsimd.dma_start(wc[:], wc_d[:])
        ws = consts.tile([108, 4], F32R, tag="ws")
        nc.gpsimd.dma_start(ws[:], ws_d[:])
        wsh = consts.tile([108, 4], F16, tag="wsh")
        nc.gpsimd.dma_start(wsh[:], ws_d[:])
        bv = consts.tile([108, 1], F32, tag="bv")
        nc.gpsimd.dma_start(bv[:], bv_d[:])
        m3 = consts.tile([32, 3], F32, tag="m3")
        nc.gpsimd.dma_start(m3[:], m3_d[:])
        warm = consts.tile([32, 3], F32, tag="warm")
        nc.scalar.activation(warm[:], m3[:], mybir.ActivationFunctionType.Exp)
        eps4 = consts.tile([4, 1], F32, tag="eps4")
        nc.vector.memset(eps4[:], 1e-8)

        dps = []
        for i in range(3):
            dp = dpads.tile([SLABP, W + 2], F16, tag=f"dp{i}")
            nc.vector.memset(dp[:], 0.0)
            dps.append(dp)

        im_p = ctx.enter_context(tc.tile_pool(name="im", bufs=1))
        # IM partition layout: p = (9*br + tap)*4 + q  (q = 16-row quarter)
        # f32r (~13-bit multiply) is plenty for both the conv and the patch
        # product; one tile serves both.
        im_r = im_p.tile([108, 16, W], F16, tag="imr")
        # 4-byte seed write: forces slot allocation before the sliced im2col
        # writes that the scheduler otherwise mis-tracks across pool scopes
        nc.gpsimd.dma_start(im_r[0:1, 0:1, 0:1], z1_d[:, :])

        # ---------------- Stage 1: softmax-expectation ----------------
        with ExitStack() as s1:
            xa_p = s1.enter_context(tc.tile_pool(name="xa", bufs=5))
            xb_p = s1.enter_context(tc.tile_pool(name="xb", bufs=5))
            ea_p = s1.enter_context(tc.tile_pool(name="ea", bufs=5))
            eb_p = s1.enter_context(tc.tile_pool(name="eb", bufs=5))
            ps_p = s1.enter_context(tc.tile_pool(name="ps1", bufs=4, space="PSUM"))
            dv_p = s1.enter_context(tc.tile_pool(name="div", bufs=3))

            for br in range(3):
                x = xs[br]
                xbf = xbs[br]
                dp = dps[br]
                for blk, (r0, nr) in enumerate(BLOCKS):
                    if nr != 32:
                        subs = [(0, nr)]
                    elif br == 0 and blk == 0:
                        # cold-start: fine-grained loads so the exp stream
                        # starts as early as possible
                        subs = [(rs, 4) for rs in range(0, 32, 4)]
                    else:
                        subs = [(0, 8), (8, 8), (16, 8), (24, 8)]
                    eas, ebs = [], []
                    for rs, ns in subs:
                        xa = xa_p.tile([128, ns, W], F16, tag="xa")
                        nc.sync.dma_start(xa[:], x[0:128, r0 + rs : r0 + rs + ns, :])
                        ea = ea_p.tile([128, ns, W], F16, tag="ea")
                        nc.scalar.activation(
                            ea[:], xa[:], mybir.ActivationFunctionType.Exp
                        )
                        eas.append(ea)

                        nh = ns // 2
                        jj0 = (r0 + rs) // 2
                        xb = xb_p.tile([128, nh, W], F16, tag="xb")
                        nc.sync.dma_start(xb[:], xbf[:, jj0 : jj0 + nh, :])
                        eb = eb_p.tile([128, nh, W], F16, tag="eb")
                        nc.scalar.activation(
                            eb[:], xb[:], mybir.ActivationFunctionType.Exp
                        )
                        ebs.append(eb)

                    ps = ps_p.tile([128, W], F32, tag="ps1")
                    n_mm = nr + nr // 2
                    sub_sz = subs[0][1]
                    k = 0
                    for r in range(nr):
                        si = r // sub_sz
                        nc.tensor.matmul(
                            ps[0:64, :],
                            la[:, r, :],
                            eas[si][:, r - subs[si][0], :],
                            start=(k == 0),
                            stop=(k == n_mm - 1),
                        )
                        k += 1
                    for j in range(nr // 2):
                        si = (2 * j) // sub_sz
                        nc.tensor.matmul(
                            ps[0:64, :],
                            lb[:, j, :],
                            ebs[si][:, j - subs[si][0] // 2, :],
                            start=(k == 0),
                            stop=(k == n_mm - 1),
                        )
                        k += 1

                    rec = dv_p.tile([32, W], F32, tag="rec")
                    nc.vector.reciprocal_approx_fast(rec[0:nr], ps[0:nr, :])
                    s1c = dv_p.tile([32, W], F32, tag="s1c")
                    nc.vector.tensor_scalar_mul(
                        s1c[0:nr], ps[32 : 32 + nr, :], m3[0:nr, blk : blk + 1]
                    )
                    dt = dv_p.tile([32, W], F16, tag="dt")
                    nc.vector.tensor_mul(dt[0:nr], s1c[0:nr], rec[0:nr])
                    nc.gpsimd.dma_start(dp[r0 : r0 + nr, 1 : W + 1], dt[0:nr])

                # patch materialization for this branch as soon as its d_pad
                # completes: br0/br1 on the otherwise-idle SWDGE queue
                # (overlaps the remaining stage-1), br2 on scalar HWDGE whose
                # exp stream has drained by then
                for tap in range(9):
                    dy, dx = tap // 3, tap % 3
                    p = (9 * br + tap) * 4
                    if br < 2:
                        eng = nc.gpsimd
                    else:
                        eng = [nc.scalar, nc.sync][tap % 2]
                    eng.dma_start(
                        im_r[p : p + 4, :, :],
                        dp[dy : dy + 64, dx : dx + W],
                    )

        # ---------------- Stage 2: affinity gates + propagation --------------
        with ExitStack() as s2:
            aff_p = s2.enter_context(tc.tile_pool(name="aff", bufs=4))
            prod_p = s2.enter_context(tc.tile_pool(name="prod", bufs=4))
            pc_p = s2.enter_context(tc.tile_pool(name="pc", bufs=3, space="PSUM"))
            pss_p = s2.enter_context(tc.tile_pool(name="pss", bufs=2, space="PSUM"))
            psa_p = s2.enter_context(tc.tile_pool(name="psa", bufs=2, space="PSUM"))
            fin_p = s2.enter_context(tc.tile_pool(name="fin", bufs=3))

            out_v = out_d.rearrange("(q n) w -> n q w", q=4)
            for n in range(16):
                pc = pc_p.tile([108, W], F32, tag="pc")
                nc.tensor.matmul(pc[:], wc[:], im_r[:, n, :], start=True, stop=True)
                aff = aff_p.tile([108, W], F16, tag="aff")
                nc.scalar.activation(
                    aff[:], pc[:], mybir.ActivationFunctionType.Relu, bias=bv[:]
                )
                pss = pss_p.tile([128, W], F32, tag="pss")
                nc.tensor.matmul(pss[0:4, :], wsh[:], aff[:], start=True, stop=True)
                prod = prod_p.tile([108, W], F32R, tag="prod")
                nc.vector.tensor_mul(prod[:], aff[:], im_r[:, n, :])
                psa = psa_p.tile([128, W], F32, tag="psa")
                nc.tensor.matmul(psa[0:4, :], ws[:], prod[:], start=True, stop=True)

                den = fin_p.tile([4, W], F32, tag="den")
                nc.scalar.activation(
                    den[:], pss[0:4, :], mybir.ActivationFunctionType.Identity,
                    bias=eps4[:],
                )
                rec2 = fin_p.tile([4, W], F32, tag="rec2")
                nc.vector.reciprocal_approx_fast(rec2[:], den[:])
                oc = fin_p.tile([4, W], F32, tag="oc")
                nc.vector.tensor_mul(oc[:], psa[0:4, :], rec2[:])
                nc.sync.dma_start(out_v[n], oc[:])

    nc.compile()
    return nc


_NC_CACHE = None


def _get_nc():
    global _NC_CACHE
    if _NC_CACHE is None:
        _NC_CACHE = _build_nc()
    return _NC_CACHE


def _host_consts(W1, g1, b1, W2, g2, b2, W3, g3, b3):
    # Stage-1 stationary weights. la[k, r, m]: row-r matmul over chunk-A
    # (d = k): col r -> 1 (s0), col 32+r -> d (s1). lb[k, j, m]: pair-j matmul
    # over chunk-B packed (k<64: d=128+k row 2j; k>=64: d=128+(k-64) row 2j+1).
    la = np.zeros((128, 32, 64), np.float32)
    for r in range(32):
        la[:, r, r] = 1.0
        la[:, r, 32 + r] = np.arange(128)
    lb = np.zeros((128, 16, 64), np.float32)
    for j in range(16):
        lb[0:64, j, 2 * j] = 1.0
        lb[0:64, j, 32 + 2 * j] = 128 + np.arange(64)
        lb[64:128, j, 2 * j + 1] = 1.0
        lb[64:128, j, 33 + 2 * j] = 128 + np.arange(64)

    # Stage-2: k/m space p = (9*br + c)*4 + q
    Ws = [W1, W2, W3]
    gs = [g1, g2, g3]
    bs = [b1, b2, b3]
    wc = np.zeros((108, 108), np.float32)
    ws = np.zeros((108, 4), np.float32)
    bv = np.zeros((108, 1), np.float32)
    for br in range(3):
        wflat = Ws[br].reshape(9, 9)  # [c, tap]
        for c in range(9):
            for tap in range(9):
                for q in range(4):
                    wc[(9 * br + tap) * 4 + q, (9 * br + c) * 4 + q] = (
                        wflat[c, tap] * gs[br][c]
                    )
        for c in range(9):
            for q in range(4):
                ws[(9 * br + c) * 4 + q, q] = 1.0
                bv[(9 * br + c) * 4 + q, 0] = bs[br][c]
    return la.astype(np.float16), lb.astype(np.float16), wc, ws, bv


def prepare_in_maps(out_1, out_2, out_3, W1, g1, b1, W2, g2, b2, W3, g3, b3):
    xs_full = [np.asarray(a, np.float32) for a in (out_1, out_2, out_3)]
    la, lb, wc, ws, bv = _host_consts(
        *[np.asarray(a, np.float32) for a in (W1, g1, b1, W2, g2, b2, W3, g3, b3)]
    )

    in_maps = []
    for c in range(N_CORES):
        b = c // 4
        h0 = SLAB * (c % 4)
        lo, hi = max(0, h0 - 1), min(H, h0 + SLAB + 1)
        mask = np.ones(SLABP, np.float32)
        if h0 == 0:
            mask[0] = 0.0
        if h0 + SLAB == H:
            mask[SLABP - 1] = 0.0
        m3 = np.zeros((32, 3), np.float32)
        for blk, (r0, nr) in enumerate(BLOCKS):
            m3[0:nr, blk] = mask[r0 : r0 + nr]

        im = {"la": la, "lb": lb, "wc": wc, "ws": ws, "bv": bv, "m3": m3,
              "z1": np.zeros((1, 1), np.float32)}
        for i, xf in enumerate(xs_full):
            shard = np.zeros((D, SLABP, W), np.float32)
            shard[:, lo - (h0 - 1) : hi - (h0 - 1), :] = xf[b, :, lo:hi, :]
            im[f"x{i + 1}"] = shard[0:128].astype(np.float16)
            # chunk-B pair packing: [p, jj, w]: p<64 -> d=128+p row 2jj,
            # p>=64 -> d=128+(p-64) row 2jj+1
            cb = shard[128:192].reshape(64, SLABP // 2, 2, W)
            im[f"xb{i + 1}"] = np.ascontiguousarray(
                np.concatenate([cb[:, :, 0, :], cb[:, :, 1, :]], axis=0)
            ).astype(np.float16)
        in_maps.append(im)
    return in_maps


def gather(results):
    out = np.zeros((B, H, W), np.float32)
    for c in range(N_CORES):
        b = c // 4
        h0 = SLAB * (c % 4)
        out[b, h0 : h0 + SLAB, :] = results[c]["out"]
    return out


def kernel(**inputs):
    in_maps = prepare_in_maps(**inputs)
    res = bass_utils.run_bass_kernel_spmd(
        _get_nc(), in_maps, core_ids=list(range(N_CORES))
    )
    return gather(res.results)


# revision 39
# speedup vs baseline: 1.0100x; 1.0100x over previous
"""DisparityFusion Trainium2 kernel (8 NeuronCores, SPMD data-parallel).

Full inputs in, full output out. Sharding: core c handles batch b=c//4 and
output rows [64*(c%4), 64*(c%4)+64), with a 1-row halo of the disparity maps
computed locally from a 1-row halo of the cost volumes (zero-padded at image
boundaries, masked after the softmax-regression).

Per-core pipeline:
  Stage 1 (per branch): softmax-expectation over D=192.
    Cost slab loaded D-on-partitions (chunk A: d 0..127; chunk B: d 128..191
    packed two rows per 128 partitions). exp on ScalarE -> bf16. Sum(e) and
    Sum(e*d) via TensorE matmuls with block-diagonal stationary bf16 weights
    into one PSUM tile per 32-row block (s0 rows 0..31, s1 rows 32..63).
    Reciprocal-multiply on VectorE, mask boundary halo rows, DMA rows into a
    zero-bordered d_pad [66, 514].
  Stage 2: 3x3 conv (1->9ch) x3 branches + BN + ReLU -> 27 affinity gates,
    CSPN abs-sum normalization folded into the final division:
      out = sum_c aff_c*patch_c / (sum_c aff_c + 1e-8)   (aff >= 0 post-ReLU)
    Patches materialized once as IM[108, 16, 512] (partition = (9br+tap)*4+q,
    q = 16-row quarter) via 27 SBUF->SBUF f32->f32r cast DMAs from d_pad; the
    conv is one [108,108] f32r matmul per 512-pixel chunk; channel sums via
    ones-pattern matmuls (S in f32r, A in bf16).
"""

import sys

sys.path.insert(0, "/opt/trn_rl_repo")

from contextlib import ExitStack

import numpy as np
import ml_dtypes

import concourse.bass as bass
import concourse.bacc as bacc
import concourse.tile as tile
from concourse import mybir
from concourse import bass_utils

B, D, H, W = 2, 192, 256, 512
N_CORES = 8
SLAB = 64            # output rows per core
SLABP = SLAB + 2     # slab + 1-row halo each side
BLOCKS = [(0, 32), (32, 32), (64, 2)]  # (r0, nr) over the 66 slab rows

F32 = mybir.dt.float32
F16 = mybir.dt.float16
F32R = mybir.dt.float32r
BF16 = mybir.dt.bfloat16
BF16_NP = ml_dtypes.bfloat16


def _build_nc():
    nc = bacc.Bacc(
        "TRN2",
        target_bir_lowering=False,
        debug=False,
        enable_asserts=False,
        num_devices=N_CORES,
    )

    xs = [
        nc.dram_tensor(f"x{i}", [128, SLABP, W], F16, kind="ExternalInput").ap()
        for i in (1, 2, 3)
    ]
    xbs = [
        nc.dram_tensor(f"xb{i}", [128, SLABP // 2, W], F16, kind="ExternalInput").ap()
        for i in (1, 2, 3)
    ]
    la_d = nc.dram_tensor("la", [128, 32, 64], F16, kind="ExternalInput").ap()
    lb_d = nc.dram_tensor("lb", [128, 16, 64], F16, kind="ExternalInput").ap()
    wc_d = nc.dram_tensor("wc", [108, 108], F32, kind="ExternalInput").ap()
    ws_d = nc.dram_tensor("ws", [108, 4], F32, kind="ExternalInput").ap()
    bv_d = nc.dram_tensor("bv", [108, 1], F32, kind="ExternalInput").ap()
    m3_d = nc.dram_tensor("m3", [32, 3], F32, kind="ExternalInput").ap()
    z1_d = nc.dram_tensor("z1", [1, 1], F32, kind="ExternalInput").ap()
    out_d = nc.dram_tensor("out", [SLAB, W], F32, kind="ExternalOutput").ap()

    with tile.TileContext(nc) as tc, ExitStack() as ctx:
        consts = ctx.enter_context(tc.tile_pool(name="consts", bufs=1))
        dpads = ctx.enter_context(tc.tile_pool(name="dpads", bufs=1))

        la = consts.tile([128, 32, 64], F16, tag="la")
        nc.gpsimd.dma_start(la[:], la_d[:])
        lb = consts.tile([128, 16, 64], F16, tag="lb")
        nc.gpsimd.dma_start(lb[:], lb_d[:])
        wc = consts.tile([108, 108], F16, tag="wc")
        nc.gpsimd.dma_start(wc[:], wc_d[:])
        ws = consts.tile([108, 4], F32R, tag="ws")
        nc.gpsimd.dma_start(ws[:], ws_d[:])
        wsh = consts.tile([108, 4], F16, tag="wsh")
        nc.gpsimd.dma_start(wsh[:], ws_d[:])
        bv = consts.tile([108, 1], F32, tag="bv")
        nc.gpsimd.dma_start(bv[:], bv_d[:])
        m3 = consts.tile([32, 3], F32, tag="m3")
        nc.gpsimd.dma_start(m3[:], m3_d[:])
        warm = consts.tile([32, 3], F32, tag="warm")
        nc.scalar.activation(warm[:], m3[:], mybir.ActivationFunctionType.Exp)
        eps4 = consts.tile([4, 1], F32, tag="eps4")
        nc.vector.memset(eps4[:], 1e-8)

        dps = []
        for i in range(3):
            dp = dpads.tile([SLABP, W + 2], F16, tag=f"dp{i}")
            nc.vector.memset(dp[:], 0.0)
            dps.append(dp)

        im_p = ctx.enter_context(tc.tile_pool(name="im", bufs=1))
        # IM partition layout: p = (9*br + tap)*4 + q  (q = 16-row quarter)
        # f32r (~13-bit multiply) is plenty for both the conv and the patch
        # product; one tile serves both.
        im_r = im_p.tile([108, 16, W], F16, tag="imr")
        # 4-byte seed write: forces slot allocation before the sliced im2col
        # writes that the scheduler otherwise mis-tracks across pool scopes
        nc.gpsimd.dma_start(im_r[0:1, 0:1, 0:1], z1_d[:, :])

        # ---------------- Stage 1: softmax-expectation ----------------
        with ExitStack() as s1:
            xa_p = s1.enter_context(tc.tile_pool(name="xa", bufs=5))
            xb_p = s1.enter_context(tc.tile_pool(name="xb", bufs=5))
            ea_p = s1.enter_context(tc.tile_pool(name="ea", bufs=5))
            eb_p = s1.enter_context(tc.tile_pool(name="eb", bufs=5))
            ps_p = s1.enter_context(tc.tile_pool(name="ps1", bufs=4, space="PSUM"))
            dv_p = s1.enter_context(tc.tile_pool(name="div", bufs=3))

            for br in range(3):
                x = xs[br]
                xbf = xbs[br]
                dp = dps[br]
                for blk, (r0, nr) in enumerate(BLOCKS):
                    if nr != 32:
                        subs = [(0, nr)]
                    elif br == 0 and blk == 0:
                        # cold-start: fine-grained loads so the exp stream
                        # starts as early as possible
                        subs = [(rs, 4) for rs in range(0, 32, 4)]
                    else:
                        subs = [(0, 8), (8, 8), (16, 8), (24, 8)]
                    eas, ebs = [], []
                    for rs, ns in subs:
                        xa = xa_p.tile([128, ns, W], F16, tag="xa")
                        nc.sync.dma_start(xa[:], x[0:128, r0 + rs : r0 + rs + ns, :])
                        ea = ea_p.tile([128, ns, W], F16, tag="ea")
                        nc.scalar.activation(
                            ea[:], xa[:], mybir.ActivationFunctionType.Exp
                        )
                        eas.append(ea)

                        nh = ns // 2
                        jj0 = (r0 + rs) // 2
                        xb = xb_p.tile([128, nh, W], F16, tag="xb")
                        nc.sync.dma_start(xb[:], xbf[:, jj0 : jj0 + nh, :])
                        eb = eb_p.tile([128, nh, W], F16, tag="eb")
                        nc.scalar.activation(
                            eb[:], xb[:], mybir.ActivationFunctionType.Exp
                        )
                        ebs.append(eb)

                    ps = ps_p.tile([128, W], F32, tag="ps1")
                    n_mm = nr + nr // 2
                    sub_sz = subs[0][1]
                    k = 0
                    for r in range(nr):
                        si = r // sub_sz
                        nc.tensor.matmul(
                            ps[0:64, :],
                            la[:, r, :],
                            eas[si][:, r - subs[si][0], :],
                            start=(k == 0),
                            stop=(k == n_mm - 1),
                        )
                        k += 1
                    for j in range(nr // 2):
                        si = (2 * j) // sub_sz
                        nc.tensor.matmul(
                            ps[0:64, :],
                            lb[:, j, :],
                            ebs[si][:, j - subs[si][0] // 2, :],
                            start=(k == 0),
                            stop=(k == n_mm - 1),
                        )
                        k += 1

                    rec = dv_p.tile([32, W], F32, tag="rec")
                    nc.vector.reciprocal_approx_fast(rec[0:nr], ps[0:nr, :])
                    s1c = dv_p.tile([32, W], F32, tag="s1c")
                    nc.vector.tensor_scalar_mul(
                        s1c[0:nr], ps[32 : 32 + nr, :], m3[0:nr, blk : blk + 1]
                    )
                    dt = dv_p.tile([32, W], F16, tag="dt")
                    nc.vector.tensor_mul(dt[0:nr], s1c[0:nr], rec[0:nr])
                    nc.gpsimd.dma_start(dp[r0 : r0 + nr, 1 : W + 1], dt[0:nr])

                # patch materialization for this branch as soon as its d_pad
                # completes: br0/br1 on the otherwise-idle SWDGE queue
                # (overlaps the remaining stage-1), br2 on scalar HWDGE whose
                # exp stream has drained by then
                for tap in range(9):
                    dy, dx = tap // 3, tap % 3
                    p = (9 * br + tap) * 4
                    if br < 2:
                        eng = nc.gpsimd
                    else:
                        eng = [nc.scalar, nc.sync][tap % 2]
                    eng.dma_start(
                        im_r[p : p + 4, :, :],
                        dp[dy : dy + 64, dx : dx + W],
                    )

        # ---------------- Stage 2: affinity gates + propagation --------------
        with ExitStack() as s2:
            aff_p = s2.enter_context(tc.tile_pool(name="aff", bufs=4))
            prod_p = s2.enter_context(tc.tile_pool(name="prod", bufs=4))
            pc_p = s2.enter_context(tc.tile_pool(name="pc", bufs=3, space="PSUM"))
            pss_p = s2.enter_context(tc.tile_pool(name="pss", bufs=2, space="PSUM"))
            psa_p = s2.enter_context(tc.tile_pool(name="psa", bufs=2, space="PSUM"))
            fin_p = s2.enter_context(tc.tile_pool(name="fin", bufs=3))

            out_v = out_d.rearrange("(q n) w -> n q w", q=4)
            for n in range(16):
                pc = pc_p.tile([108, W], F32, tag="pc")
                nc.tensor.matmul(pc[:], wc[:], im_r[:, n, :], start=True, stop=True)
                aff = aff_p.tile([108, W], F16, tag="aff")
                nc.scalar.activation(
                    aff[:], pc[:], mybir.ActivationFunctionType.Relu, bias=bv[:]
                )
                pss = pss_p.tile([128, W], F32, tag="pss")
                nc.tensor.matmul(pss[0:4, :], wsh[:], aff[:], start=True, stop=True)
                prod = prod_p.tile([108, W], F32R, tag="prod")
                nc.vector.tensor_mul(prod[:], aff[:], im_r[:, n, :])
                psa = psa_p.tile([128, W], F32, tag="psa")
                nc.tensor.matmul(psa[0:4, :], ws[:], prod[:], start=True, stop=True)

                den = fin_p.tile([4, W], F32, tag="den")
                nc.scalar.activation(
                    den[:], pss[0:4, :], mybir.ActivationFunctionType.Identity,
                    bias=eps4[:],
                )
                rec2 = fin_p.tile([4, W], F32, tag="rec2")
                nc.vector.reciprocal_approx_fast(rec2[:], den[:])
                oc = fin_p.tile([4, W], F32, tag="oc")
                nc.vector.tensor_mul(oc[:], psa[0:4, :], rec2[:])
                nc.sync.dma_start(out_v[n], oc[:])

    nc.compile()
    return nc


_NC_CACHE = None


def _get_nc():
    global _NC_CACHE
    if _NC_CACHE is None:
        _NC_CACHE = _build_nc()
    return _NC_CACHE


def _host_consts(W1, g1, b1, W2, g2, b2, W3, g3, b3):
    # Stage-1 stationary weights. la[k, r, m]: row-r matmul over chunk-A
    # (d = k): col r -> 1 (s0), col 32+r -> d (s1). lb[k, j, m]: pair-j matmul
    # over chunk-B packed (k<64: d=128+k row 2j; k>=64: d=128+(k-64) row 2j+1).
    la = np.zeros((128, 32, 64), np.float32)
    for r in range(32):
        la[:, r, r] = 1.0
        la[:, r, 32 + r] = np.arange(128)
    lb = np.zeros((128, 16, 64), np.float32)
    for j in range(16):
        lb[0:64, j, 2 * j] = 1.0
        lb[0:64, j, 32 + 2 * j] = 128 + np.arange(64)
        lb[64:128, j, 2 * j + 1] = 1.0
        lb[64:128, j, 33 + 2 * j] = 128 + np.arange(64)

    # Stage-2: k/m space p = (9*br + c)*4 + q
    Ws = [W1, W2, W3]
    gs = [g1, g2, g3]
    bs = [b1, b2, b3]
    wc = np.zeros((108, 108), np.float32)
    ws = np.zeros((108, 4), np.float32)
    bv = np.zeros((108, 1), np.float32)
    for br in range(3):
        wflat = Ws[br].reshape(9, 9)  # [c, tap]
        for c in range(9):
            for tap in range(9):
                for q in range(4):
                    wc[(9 * br + tap) * 4 + q, (9 * br + c) * 4 + q] = (
                        wflat[c, tap] * gs[br][c]
                    )
        for c in range(9):
            for q in range(4):
                ws[(9 * br + c) * 4 + q, q] = 1.0
                bv[(9 * br + c) * 4 + q, 0] = bs[br][c]
    return la.astype(np.float16), lb.astype(np.float16), wc, ws, bv


def prepare_in_maps(out_1, out_2, out_3, W1, g1, b1, W2, g2, b2, W3, g3, b3):
    xs_full = [np.asarray(a, np.float32) for a in (out_1, out_2, out_3)]
    la, lb, wc, ws, bv = _host_consts(
        *[np.asarray(a, np.float32) for a in (W1, g1, b1, W2, g2, b2, W3, g3, b3)]
    )

    in_maps = []
    for c in range(N_CORES):
        b = c // 4
        h0 = SLAB * (c % 4)
        lo, hi = max(0, h0 - 1), min(H, h0 + SLAB + 1)
        mask = np.ones(SLABP, np.float32)
        if h0 == 0:
            mask[0] = 0.0
        if h0 + SLAB == H:
            mask[SLABP - 1] = 0.0
        m3 = np.zeros((32, 3), np.float32)
        for blk, (r0, nr) in enumerate(BLOCKS):
            m3[0:nr, blk] = mask[r0 : r0 + nr]

        im = {"la": la, "lb": lb, "wc": wc, "ws": ws, "bv": bv, "m3": m3,
              "z1": np.zeros((1, 1), np.float32)}
        for i, xf in enumerate(xs_full):
            shard = np.zeros((D, SLABP, W), np.float32)
            shard[:, lo - (h0 - 1) : hi - (h0 - 1), :] = xf[b, :, lo:hi, :]
            im[f"x{i + 1}"] = shard[0:128].astype(np.float16)
            # chunk-B pair packing: [p, jj, w]: p<64 -> d=128+p row 2jj,
            # p>=64 -> d=128+(p-64) row 2jj+1
            cb = shard[128:192].reshape(64, SLABP // 2, 2, W)
            im[f"xb{i + 1}"] = np.ascontiguousarray(
                np.concatenate([cb[:, :, 0, :], cb[:, :, 1, :]], axis=0)
            ).astype(np.float16)
        in_maps.append(im)
    return in_maps


def gather(results):
    out = np.zeros((B, H, W), np.float32)
    for c in range(N_CORES):
        b = c // 4
        h0 = SLAB * (c % 4)
        out[b, h0 : h0 + SLAB, :] = results[c]["out"]
    return out


def kernel(**inputs):
    in_maps = prepare_in_maps(**inputs)
    res = bass_utils.run_bass_kernel_spmd(
        _get_nc(), in_maps, core_ids=list(range(N_CORES))
    )
    return gather(res.results)
